# revision 38
# baseline (speedup 1.0000x reference)
"""Batch per-sample 3x3 conv (B=32, C=32, H=W=256, pad=1) on 8 TRN2 cores.

Data parallel: 4 samples per core. The PE array is split into 16
independent 32x32 tiles (tile_position): tile (r, c) convolves sample r
(contraction = its 32 input channels, on SBUF partitions 32r..32r+31)
for output-row-pair chunk c, writing psum bank r at partitions
32c..32c+31. All 16 tiles stream concurrently (4 moving XBUS streams, 4
col groups), 4x the useful MACs/cycle of a 128x128 block-diagonal
stationary; the sustained rate is set by the per-tile matmul turnaround
(~550ns per N=512 matmul) across 16 tiles, ~34ns/matmul aggregate.
Spatial taps are access-pattern offsets into a fully SBUF-resident
padded x image; dx=+-1 taps write partial-width psum slices. fp16 in /
fp16 out halves HBM traffic vs f32; PSUM accumulates fp32 and
ScalarE/VectorE split the bias-fused drain into a staging tile shared
by two sweeps, shipped as a single contiguous output DMA on the scalar
HWDGE ring (keeps trigger cost off the critical path). The host
unscrambles the psum-bank output layout with one transpose.
"""

import numpy as np

N_CORES = 8
B, C_IN, C_OUT, H, W, KS = 32, 32, 32, 256, 256, 3
SPC = B // N_CORES  # samples per core
NCG = 4  # PE column groups = row-pair chunks per sweep
SW = 2 * NCG  # output rows per sweep
NSW = H // SW  # sweeps

_CACHE = {}


def _dedup_ldweights(nc):
    """Drop InstLdweights that reload the identical weights into the same
    PE tile as the previous load (no waits/updates): the weights are
    already in the array, the reload is pure overhead."""
    dropped = 0
    for f in nc.m.functions:
        for bb in f.blocks:
            insts = bb.instructions
            last = {}
            keep = []
            changed = False
            for i in insts:
                if type(i).__name__ == "InstLdweights":
                    sig = i.concise().split("in=", 1)[-1]
                    tp = sig.rsplit("tile_position", 1)[-1]
                    if (
                        not i.has_wait()
                        and not i.has_update()
                        and last.get(tp) == sig
                    ):
                        dropped += 1
                        changed = True
                        continue
                    last[tp] = sig
                keep.append(i)
            if changed:
                bb.instructions = keep
    return dropped


def _build():
    import concourse.bacc as bacc
    import concourse.mybir as mybir
    import concourse.tile as tile

    f32 = mybir.dt.float32
    f16 = mybir.dt.float16
    u16 = mybir.dt.uint16

    nc = bacc.Bacc(
        "TRN2", target_bir_lowering=False, debug=False, num_devices=N_CORES
    )
    x_d = nc.dram_tensor("x", [128, H, W], f16, kind="ExternalInput").ap()
    w_d = nc.dram_tensor("w", [128, 9, 32], f16, kind="ExternalInput").ap()
    bias_d = nc.dram_tensor("bias_v", [128, SPC], f32, kind="ExternalInput").ap()
    o_d = nc.dram_tensor(
        "out", [NSW // 2, 128, 2, SPC, 2, W], f16, kind="ExternalOutput"
    ).ap()

    with tile.TileContext(nc) as tc:
        with (
            tc.tile_pool(name="const", bufs=1) as cpool,
            tc.tile_pool(name="xp", bufs=1) as xpool,
            tc.tile_pool(name="op", bufs=4) as opool,
            tc.tile_pool(name="ps", bufs=2, space="PSUM") as ppool,
        ):
            # warm scratch first: gpsimd memset gates the PE warm chain, so
            # it runs before anything else queues on that engine
            dumw = cpool.tile([128, 640], f16)
            nc.gpsimd.memset(dumw[:].bitcast(u16), 0)
            # weights/bias on the gpsimd (SWDGE) queue so the sync queue is
            # free for the first x pieces
            w_sb = cpool.tile([128, 9, 32], f16)
            nc.gpsimd.dma_start(out=w_sb[:, :, :], in_=w_d[:, :, :])
            b_sb = cpool.tile([128, SPC], f32)
            nc.gpsimd.dma_start(out=b_sb[:, :], in_=bias_d[:, :])

            # padded x image, fully SBUF-resident: row 0 / row H+1 are the
            # zero-pad rows, rows 1..H stream in from HBM in pieces (one
            # HWDGE ring serializes pieces with ~µs completion-receipt gaps,
            # so both rings carry a share)
            xb = xpool.tile([128, H + 2, W], f16, tag="xb", name="xb")
            nc.vector.memset(xb[:, 0, :].bitcast(u16), 0)
            nc.vector.memset(xb[:, H + 1, :].bitcast(u16), 0)
            # both HWDGE rings feed x: sync takes the head and back half,
            # scalar takes an early middle chunk (it completes before the
            # first output pair queues on that ring); SWDGE (gpsimd) is too
            # slow for bulk x
            sync_pieces = [(0, 4), (4, 12), (12, 28)] + [
                (k, min(k + 16, H)) for k in range(124, H, 16)
            ]
            scal_pieces = [(28, 52), (52, 76), (76, 100), (100, 124)]
            ring_plan = [(nc.sync, sync_pieces), (nc.scalar, scal_pieces)]
            for ring, pieces in ring_plan:
                for a, b in pieces:
                    ring.dma_start(out=xb[:, 1 + a : 1 + b, :], in_=x_d[:, a:b, :])

            # warm the PE clock (HAM gate) during the initial x DMA wait:
            # dummy matmuls on a zeroed scratch tile, no data dependencies
            psw = ppool.tile([128, 2, W], f32, tag="ps0", name="psw")
            NWARM = 12
            for k in range(NWARM):
                nc.tensor.matmul(
                    psw[:, :, :],
                    dumw[:, 0:128],
                    dumw[:, 128:640],
                    start=(k == 0),
                    stop=(k == NWARM - 1),
                )

            # tap order: dx=0 taps first so the start=True matmul covers the
            # full psum region (dx=+-1 taps write partial-width slices)
            TAPS = [(dy, 0) for dy in (-1, 0, 1)] + [
                (dy, dx) for dx in (-1, 1) for dy in (-1, 0, 1)
            ]

            for s in range(NSW):
                pss = [
                    ppool.tile([128, 2, W], f32, tag=f"ps{r}", name=f"ps{r}")
                    for r in range(SPC)
                ]
                for ti in range(9):
                    dy, dx = TAPS[ti]
                    tap = (dy + 1) * 3 + (dx + 1)
                    # out col w <- x col w+dx; clip to the image border
                    xa, ow = max(dx, 0), max(-dx, 0)
                    n = W - abs(dx)
                    for r in range(SPC):
                        for c in range(NCG):
                            rs = SW * s + 2 * c + dy + 1
                            nc.tensor.matmul(
                                pss[r][32 * c : 32 * c + 32, :, ow : ow + n],
                                w_sb[32 * r : 32 * r + 32, tap, :],
                                xb[32 * r : 32 * r + 32, rs : rs + 2, xa : xa + n],
                                start=(ti == 0),
                                stop=(ti == 8),
                                tile_position=(32 * r, 32 * c),
                            )
                # drain all 4 banks into one staging tile (bias fused, fp16);
                # ScalarE and VectorE split the banks (parallel PSUM access on
                # distinct banks). Two sweeps share one staging tile and ship
                # as a single contiguous DMA on the scalar HWDGE ring (halves
                # the per-DMA trigger + completion-receipt overhead, and that
                # ring never queues behind the x stream); the final pair goes
                # out per sweep-half so the tail DMA starts sooner
                if s % 2 == 0:
                    ob = opool.tile(
                        [128, 2, SPC, 2, W], f16, tag="ob", name="ob"
                    )
                for r in range(SPC):
                    if r < 2:
                        nc.scalar.add(
                            out=ob[:, s % 2, r, :, :],
                            in_=pss[r][:, :, :],
                            add=b_sb[:, r : r + 1],
                        )
                    else:
                        nc.vector.tensor_scalar_add(
                            ob[:, s % 2, r, :, :],
                            pss[r][:, :, :],
                            b_sb[:, r : r + 1],
                        )
                if s == NSW - 2:
                    nc.scalar.dma_start(
                        out=o_d[s // 2, :, 0, :, :, :], in_=ob[:, 0, :, :, :]
                    )
                elif s == NSW - 1:
                    nc.scalar.dma_start(
                        out=o_d[s // 2, :, 1, :, :, :], in_=ob[:, 1, :, :, :]
                    )
                elif s % 2 == 1:
                    nc.scalar.dma_start(
                        out=o_d[s // 2, :, :, :, :, :], in_=ob[:, :, :, :, :]
                    )

    _dedup_ldweights(nc)
    nc.compile()
    return nc


def _get_nc():
    if "nc" not in _CACHE:
        _CACHE["nc"] = _build()
    return _CACHE["nc"]


def _shard_inputs(x, weight, bias):
    x = np.asarray(x, dtype=np.float32)
    weight = np.asarray(weight, dtype=np.float32)
    bias = np.asarray(bias, dtype=np.float32)
    in_maps = []
    for core in range(N_CORES):
        sl = slice(SPC * core, SPC * (core + 1))
        xs = np.ascontiguousarray(x[sl]).reshape(128, H, W).astype(np.float16)
        # [s, co, ci, ky, kx] -> [s, ci, (ky kx), co]; partition 32s+ci
        ws = np.ascontiguousarray(
            weight[sl].transpose(0, 2, 3, 4, 1).reshape(128, 9, 32)
        ).astype(np.float16)
        # drain bias vector: partition 32c+co of bank r needs bias[r, co]
        bs = np.ascontiguousarray(np.tile(bias[sl].T, (NCG, 1)))  # [128, SPC]
        in_maps.append({"x": xs, "w": ws, "bias_v": bs})
    return in_maps


def run(x, weight, bias, trace=False):
    from concourse.bass_utils import run_bass_kernel_spmd

    nc = _get_nc()
    in_maps = _shard_inputs(x, weight, bias)
    res = run_bass_kernel_spmd(
        nc, in_maps, core_ids=list(range(N_CORES)), trace=trace
    )
    out = np.empty((B, C_OUT, H, W), dtype=np.float32)
    for core in range(N_CORES):
        # [s2, c, co, p2, r, rr, w] -> [r, co, (s2 p2 c rr), w]
        arr = res.results[core]["out"].reshape(NSW // 2, NCG, 32, 2, SPC, 2, W)
        arr = arr.transpose(4, 2, 0, 3, 1, 5, 6).reshape(SPC, C_OUT, H, W)
        out[SPC * core : SPC * (core + 1)] = arr.astype(np.float32)
    return out, res


def kernel(x, weight, bias):
    out, _ = run(x, weight, bias, trace=False)
    return out
